# revision 1
# baseline (speedup 1.0000x reference)
"""AnchorTargetLayer max-IoU kernel for 8 TRN2 NeuronCores (v5, fp16).

max_iou[b, n] = max_g IoU(anchor_n, gt_box[b, g]);
anchors [100000, 4] f32, gt_boxes [4, 64, 4] f32 -> out [4, 100000] f32.

Sharding: anchors split 8 ways (12544/core incl pad), gt replicated, no
collectives. Per-core layout: anchors on SBUF partitions (128/block, 98
blocks), all B*G = 256 (batch, gt) pairs on the free dim, batch-major.

Coordinates pre-scaled by 1/16 on the host, GT rows cast to fp16 (anchor
per-partition scalars stay f32 as the ISA requires); chain err ~3.5e-3
vs the 2e-2 budget.

Engine facts this schedule is built on (measured on HW):
 - DVE tensor_scalar with an SBUF scalar hits the 4x fp16 perf mode
   (~275ns at 256 wide); scalar_tensor_tensor has NO fast uop (1x).
 - GpSimd (Pool) shares an SBUF port with DVE: any Pool op stalls
   concurrent 2-port DVE ops to Pool's (slow, 2.5cyc/elem) duration, so
   Pool is left COMPLETELY IDLE.
 - ACT (Scalar) runs independently; narrow ops cost ~224cyc bubble.

Per (anchor, pair), fp16:
  t = min(g2, a2), m = min(-g1, -a1)   4 TS-ptr (4x) per block, x|y
  s = t + m                            wide DVE TT 2x (both axes at once)
  sxr = relu(s_x)                      ACT wide relu
  int = sxr * s_y                      wide DVE TT (s_y relu deferred)
  sa  = garea + areaA                  ACT Identity+bias narrow
  rs  = 1/sa                           ACT Reciprocal wide
  w   = int * rs                       wide DVE TT
  red = max over g:64->32->16 pairwise DVE TT + tensor_reduce over 16
Final: v = relu(vout); iou = v / (1 - v).

Superblocks of C=14 blocks; S/RS double-buffered across DVE<->ACT; the
consumer chain for superblock s-1 runs on DVE after front(s) so ACT has
a full superblock of latency slack.
"""

import os
import sys

import numpy as np

sys.path.insert(0, "/opt/trn_rl_repo")

import concourse.bass as bass
import concourse.mybir as mybir
from concourse.bass_utils import run_bass_kernel_spmd

N_ANCHORS = 100000
BATCH = 4
N_GT = 64
N_CORES = 8

P = 128
BLOCKS = 98
C = 14                      # blocks per superblock
NSB = BLOCKS // C           # 7 superblocks
N_LOC = P * BLOCKS          # 12544
N_PAD = N_LOC * N_CORES     # 100352
NPAIR = BATCH * N_GT        # 256
SEG = C * BATCH             # 56 (block, batch) rows per superblock
NB = 2                      # S/RS double-buffer depth

F32 = mybir.dt.float32
F16 = mybir.dt.float16
COORD_SCALE = 1.0 / 16.0

LAST_EXEC_NS = None


def _ensure_axon_ntff_hook():
    try:
        import antenv.axon_hooks  # noqa: F401

        return
    except ImportError:
        pass
    import contextlib
    import ctypes
    import types

    import antenv

    m = types.ModuleType("antenv.axon_hooks")
    m._hook = None

    def set_axon_ntff_profile_hook(h):
        m._hook = h

    def get_axon_ntff_profile_hook():
        return m._hook

    m.set_axon_ntff_profile_hook = set_axon_ntff_profile_hook
    m.get_axon_ntff_profile_hook = get_axon_ntff_profile_hook
    sys.modules["antenv.axon_hooks"] = m
    antenv.axon_hooks = m

    so_path = os.environ.get("PJRT_LIBRARY_PATH", "/opt/axon/libaxon_pjrt.so")
    try:
        lib = ctypes.CDLL(so_path)
    except OSError:
        return
    if not hasattr(lib, "axon_start_nrt_profile"):
        return
    lib.axon_start_nrt_profile.argtypes = [
        ctypes.POINTER(ctypes.c_int64),
        ctypes.c_size_t,
    ]
    lib.axon_start_nrt_profile.restype = ctypes.c_int64
    lib.axon_stop_nrt_profile.argtypes = [ctypes.c_char_p]
    lib.axon_stop_nrt_profile.restype = ctypes.c_int64

    @contextlib.contextmanager
    def _hook(output_dir, device_ids):
        import jax

        jax.devices()
        if device_ids:
            ids = (ctypes.c_int64 * len(device_ids))(*device_ids)
            rc = lib.axon_start_nrt_profile(ids, len(device_ids))
        else:
            rc = lib.axon_start_nrt_profile(None, 0)
        if rc != 0:
            raise RuntimeError(f"axon_start_nrt_profile rc={rc}")
        try:
            yield
        finally:
            n = lib.axon_stop_nrt_profile(str(output_dir).encode())
            if n < 0:
                raise RuntimeError(f"axon_stop_nrt_profile rc={n}")

    set_axon_ntff_profile_hook(_hook)


def _patch_upload_artifacts():
    import concourse.bass_utils as bu

    if getattr(bu.upload_artifacts, "_safe", False):
        return
    orig = bu.upload_artifacts

    def safe(tmpdir):
        try:
            return orig(tmpdir)
        except Exception:
            return tmpdir

    safe._safe = True
    bu.upload_artifacts = safe


def _act_recip(scalar_eng, nc, out_ap, in_ap, bias=0.0, scale=1.0):
    """Directly emit Activation(Reciprocal) (the nc.scalar.activation wrapper
    rejects Reciprocal)."""
    ins = [scalar_eng.lower_ap(in_ap)]
    for argv in (bias, scale, 0.0):  # bias, scale, alpha
        ins.append(mybir.ImmediateValue(dtype=F32, value=argv))
    return scalar_eng.add_instruction(
        mybir.InstActivation(
            name=nc.get_next_instruction_name(),
            func=mybir.ActivationFunctionType.Reciprocal,
            ins=ins,
            outs=[scalar_eng.lower_ap(out_ap)],
        )
    )


def _build_graph():
    nc = bass.Bass()
    ASC_ext = nc.declare_dram_parameter("ascal", [P, BLOCKS * 4], F32, isOutput=False)
    AR_ext = nc.declare_dram_parameter("aarea", [P, BLOCKS], F32, isOutput=False)
    GT_ext = nc.declare_dram_parameter("gtrows", [5, NPAIR], F16, isOutput=False)
    out_ext = nc.declare_dram_parameter("out", [P, BLOCKS * 4], F32, isOutput=True)

    Alu = mybir.AluOpType
    from contextlib import ExitStack

    with ExitStack() as _st:
        e = _st.enter_context
        ASC = e(nc.sbuf_tensor("ASC", [P, BLOCKS * 4], F32))
        AR = e(nc.sbuf_tensor("AR", [P, BLOCKS], F32))
        GTB = e(nc.sbuf_tensor("GTB", [P, 5, NPAIR], F16))
        T = e(nc.sbuf_tensor("T", [P, 2, C, NPAIR], F16))
        M = e(nc.sbuf_tensor("M", [P, 2, C, NPAIR], F16))
        S = e(nc.sbuf_tensor("S", [P, NB, 2, C, NPAIR], F16))
        SXR = e(nc.sbuf_tensor("SXR", [P, NB, C, NPAIR], F16))
        INT = e(nc.sbuf_tensor("INT", [P, C, NPAIR], F16))
        SA = e(nc.sbuf_tensor("SA", [P, NB, C, NPAIR], F16))
        RS = e(nc.sbuf_tensor("RS", [P, NB, C, NPAIR], F16))
        W = e(nc.sbuf_tensor("W", [P, SEG, N_GT], F16))
        M1 = e(nc.sbuf_tensor("M1", [P, SEG, 32], F16))
        M2 = e(nc.sbuf_tensor("M2", [P, SEG, 16], F16))
        VOUT = e(nc.sbuf_tensor("VOUT", [P, BLOCKS * 4], F32))
        V2 = e(nc.sbuf_tensor("V2", [P, BLOCKS * 4], F32))
        R1 = e(nc.sbuf_tensor("R1", [P, BLOCKS * 4], F32))
        MIOU = e(nc.sbuf_tensor("MIOU", [P, BLOCKS * 4], F32))
        block = e(nc.Block())
        dma_sem = e(nc.semaphore("dma_sem"))
        dve_sem = e(nc.semaphore("dve_sem"))
        act_sem = e(nc.semaphore("act_sem"))

        GX2 = GTB[:, 0, :]
        GX1N = GTB[:, 1, :]
        GY2 = GTB[:, 2, :]
        GY1N = GTB[:, 3, :]
        GAREA = GTB[:, 4, :]

        @block.sync
        def _(sync):
            sync.dma_start(out=ASC[:, :], in_=ASC_ext[:, :]).then_inc(dma_sem, 16)
            sync.dma_start(out=AR[:, :], in_=AR_ext[:, :]).then_inc(dma_sem, 16)
            g_ap = GT_ext[:, :]
            g_b = bass.AP(
                tensor=g_ap.tensor, offset=g_ap.offset, ap=[[0, P]] + list(g_ap.ap)
            )
            sync.dma_start(out=GTB[:, :, :], in_=g_b).then_inc(dma_sem, 16)

        # sem targets: dve: s+1 after S(s); NSB+1 v2; NSB+2 miou
        #              act: 2s+1 sxr(s); 2s+2 rs(s); 2NSB+1 r1
        def dve_tail(vector, s, self_sxr=False):
            """Consumer chain for superblock s: int, w, pairwise maxes,
            reduce. Runs on DVE one superblock behind the front (except the
            last superblock, which computes its own relu to avoid waiting
            for ACT)."""
            sb = s % NB
            if self_sxr:
                vector.tensor_scalar(
                    out=SXR[:, sb, :, :], in0=S[:, sb, 0, :, :], scalar1=0.0,
                    scalar2=None, op0=Alu.max,
                )
            else:
                vector.wait_ge(act_sem, 2 * s + 2)  # sxr(s)
            vector.tensor_tensor(
                out=INT[:, :, :], in0=SXR[:, sb, :, :], in1=S[:, sb, 1, :, :],
                op=Alu.mult,
            )
            vector.wait_ge(act_sem, 2 * s + 1)  # rs(s)
            vector.tensor_tensor(
                out=W[:, :, :],
                in0=INT[:, :, :].rearrange("p c (bt g) -> p (c bt) g", bt=BATCH),
                in1=RS[:, sb, :, :].rearrange("p c (bt g) -> p (c bt) g", bt=BATCH),
                op=Alu.mult,
            )
            vector.tensor_tensor(
                out=M1[:, :, :], in0=W[:, :, 0:32], in1=W[:, :, 32:64], op=Alu.max
            )
            vector.tensor_tensor(
                out=M2[:, :, :], in0=M1[:, :, 0:16], in1=M1[:, :, 16:32], op=Alu.max
            )
            vector.tensor_reduce(
                out=VOUT[:, s * SEG : (s + 1) * SEG],
                in_=M2[:, :, :],
                axis=mybir.AxisListType.X,
                op=Alu.max,
            )

        @block.vector
        def _(vector):
            vector.wait_ge(dma_sem, 48)
            for s in range(NSB):
                sb = s % NB
                for j in range(C):
                    blk = s * C + j
                    ax2 = ASC[:, 4 * blk + 0 : 4 * blk + 1]
                    nax1 = ASC[:, 4 * blk + 1 : 4 * blk + 2]
                    ay2 = ASC[:, 4 * blk + 2 : 4 * blk + 3]
                    nay1 = ASC[:, 4 * blk + 3 : 4 * blk + 4]
                    vector.tensor_scalar(
                        out=T[:, 0, j, :], in0=GX2, scalar1=ax2,
                        scalar2=None, op0=Alu.min,
                    )
                    vector.tensor_scalar(
                        out=T[:, 1, j, :], in0=GY2, scalar1=ay2,
                        scalar2=None, op0=Alu.min,
                    )
                    vector.tensor_scalar(
                        out=M[:, 0, j, :], in0=GX1N, scalar1=nax1,
                        scalar2=None, op0=Alu.min,
                    )
                    vector.tensor_scalar(
                        out=M[:, 1, j, :], in0=GY1N, scalar1=nay1,
                        scalar2=None, op0=Alu.min,
                    )
                if s >= 1:
                    dve_tail(vector, s - 1)
                # S slot: ACT sxr(s-NB) must have read it (int(s-NB) already
                # done on this engine via the tail above)
                if s >= NB:
                    vector.wait_ge(act_sem, 2 * (s - NB) + 2)
                vector.tensor_tensor(
                    out=S[:, sb, :, :, :], in0=T[:, :, :, :], in1=M[:, :, :, :],
                    op=Alu.add,
                ).then_inc(dve_sem, 1)
            dve_tail(vector, NSB - 1, self_sxr=True)
            # final: v = relu(vout); iou = v * (1 / (1 - v))
            vector.tensor_scalar(
                out=V2[:, :], in0=VOUT[:, :], scalar1=0.0, scalar2=None, op0=Alu.max
            ).then_inc(dve_sem, 1)
            vector.wait_ge(act_sem, 2 * NSB + 1)
            vector.tensor_tensor(
                out=MIOU[:, :], in0=V2[:, :], in1=R1[:, :], op=Alu.mult
            ).then_inc(dve_sem, 1)

        @block.scalar
        def _(scalar):
            scalar.wait_ge(dma_sem, 48)

            def sa_batch(s):
                for j in range(C):
                    blk = s * C + j
                    scalar.activation(
                        out=SA[:, s % NB, j, :], in_=GAREA,
                        func=mybir.ActivationFunctionType.Identity,
                        bias=AR[:, blk : blk + 1], scale=1.0,
                    )

            sa_batch(0)
            for s in range(NSB):
                sb = s % NB
                # rs(s) only needs SA(s) (done an iteration ago); dve >= s
                # means w(s-2) has read the RS slot being overwritten
                if s >= NB:
                    scalar.wait_ge(dve_sem, s)
                _act_recip(
                    scalar, nc, RS[:, sb, :, :], SA[:, sb, :, :]
                ).then_inc(act_sem, 1)
                # dve >= s+1 means S(s) done; it also means int(s-2) is done
                # (it precedes S(s) on DVE), freeing the SXR slot
                if s + 1 < NSB:
                    scalar.wait_ge(dve_sem, s + 1)
                    scalar.activation(
                        out=SXR[:, sb, :, :], in_=S[:, sb, 0, :, :],
                        func=mybir.ActivationFunctionType.Relu,
                    ).then_inc(act_sem, 1)
                    sa_batch(s + 1)
                else:
                    # DVE computes its own relu for the last superblock;
                    # keep the act tick numbering with a tiny copy
                    scalar.activation(
                        out=SA[:, sb, 0, 0:1], in_=SA[:, sb, 0, 0:1],
                        func=mybir.ActivationFunctionType.Identity,
                    ).then_inc(act_sem, 1)
            scalar.wait_ge(dve_sem, NSB + 1)
            _act_recip(
                scalar, nc, R1[:, :], V2[:, :], bias=1.0, scale=-1.0
            ).then_inc(act_sem, 1)

        @block.sync
        def _(sync):
            sync.wait_ge(dve_sem, NSB + 2)
            sync.dma_start(out=out_ext[:, :], in_=MIOU[:, :]).then_inc(dma_sem, 16)
            sync.wait_ge(dma_sem, 64)

    return nc


def kernel(anchors: np.ndarray, gt_boxes: np.ndarray) -> np.ndarray:
    global LAST_EXEC_NS
    anchors = np.asarray(anchors, dtype=np.float32) * COORD_SCALE
    gt_boxes = np.asarray(gt_boxes, dtype=np.float32) * COORD_SCALE

    apad = np.zeros((N_PAD, 4), dtype=np.float32)
    apad[:N_ANCHORS] = anchors

    g = gt_boxes.reshape(NPAIR, 4).astype(np.float32)
    garea = (g[:, 2] - g[:, 0]) * (g[:, 3] - g[:, 1])
    gtrows = np.stack([g[:, 2], -g[:, 0], g[:, 3], -g[:, 1], garea])
    gtrows = np.ascontiguousarray(gtrows.astype(np.float16))

    in_maps = []
    for c in range(N_CORES):
        sh = apad[c * N_LOC : (c + 1) * N_LOC]
        a3 = sh.reshape(P, BLOCKS, 4)
        asc = np.empty_like(a3)
        asc[:, :, 0] = a3[:, :, 2]   # ax2
        asc[:, :, 1] = -a3[:, :, 0]  # -ax1
        asc[:, :, 2] = a3[:, :, 3]   # ay2
        asc[:, :, 3] = -a3[:, :, 1]  # -ay1
        aarea = (a3[:, :, 2] - a3[:, :, 0]) * (a3[:, :, 3] - a3[:, :, 1])
        in_maps.append(
            {
                "ascal": np.ascontiguousarray(asc.reshape(P, BLOCKS * 4)),
                "aarea": np.ascontiguousarray(aarea.astype(np.float32)),
                "gtrows": gtrows,
            }
        )

    nc = _build_graph()
    trace = os.environ.get("ANCHOR_TRACE", "0") == "1"
    core_ids = list(range(N_CORES))
    if trace:
        _ensure_axon_ntff_hook()
        _patch_upload_artifacts()
        try:
            res = run_bass_kernel_spmd(nc, in_maps, core_ids=core_ids, trace=True)
        except Exception as e:
            print(f"trace run failed ({type(e).__name__}: {e}); falling back", file=sys.stderr)
            res = run_bass_kernel_spmd(nc, in_maps, core_ids=core_ids, trace=False)
    else:
        res = run_bass_kernel_spmd(nc, in_maps, core_ids=core_ids, trace=False)
    LAST_EXEC_NS = res.exec_time_ns

    out = np.empty((BATCH, N_PAD), dtype=np.float32)
    for c in range(N_CORES):
        o = res.results[c]["out"].reshape(P, BLOCKS, 4)
        out[:, c * N_LOC : (c + 1) * N_LOC] = o.transpose(2, 0, 1).reshape(BATCH, N_LOC)
    return out[:, :N_ANCHORS]



# revision 4
# speedup vs baseline: 1.4553x; 1.4553x over previous
"""AnchorTargetLayer max-IoU kernel for 8 TRN2 NeuronCores (v7).

max_iou[b, n] = max_g IoU(anchor_n, gt_box[b, g]);
anchors [100000, 4] f32, gt_boxes [4, 64, 4] f32 -> out [4, 100000] f32.

Sharding: anchors split 8 ways (12544/core incl pad), gt replicated, no
collectives.

v7 layout (vs v5's anchors-on-partitions): (batch, gt) pairs live on
PARTITIONS — p = b*32 + gh, gl in {0,1} on the free dim (g = gh*2+gl) —
and ANCHORS live on the FREE dim, broadcast to all 128 partitions by a
partition-stride-0 DMA (4.1MB/chunk, ~10.4us, fully overlapped). This
turns v5's 392 narrow (256-wide) tensor_scalar ops (~360ns each, fixed
overhead dominated) into a handful of wide (3136) ops per chunk.

Per chunk (Fc=3136 anchors, 4 chunks/core), all fp16:
  clip:  CHL[ax,gl] = clamp(a_lo/a_hi rows; g1[p], g2[p])   4 TS 2-op @4x
  span:  I = CHL[...,hi] - CHL[...,lo]  (>=0 by clipping)   1 wide TT
  int:   INT = Ix * Iy                                      1 TT
  w:     W = INT * RS,  RS = 1/(aarea + garea) from ACT     1 TT
  glmax: R0 = max(W[gl0], W[gl1])                           1 TT
  gmax over gh=32 (partitions): stream_transpose 32x32 puts gh
         innermost in the free dim, then a 5-step pairwise TT-max tree.
Final: iou = v / (1 - v)  (w = int/(areaA+areaG) is monotone in iou).

Engine split: DVE ~29us/chunk; ACT runs the two reciprocals per chunk
(bias = garea per partition, fused add) ~6us; DMA broadcast ~10.4us.
SBUF: CHL block is reused for INT/W/R0/ST/tree scratch (serial chain).
"""

import os
import sys

import numpy as np

sys.path.insert(0, "/opt/trn_rl_repo")

import concourse.bass as bass
import concourse.mybir as mybir
from concourse.bass_utils import run_bass_kernel_spmd

N_ANCHORS = 100000
BATCH = 4
N_GT = 64
N_CORES = 8

P = 128
FC = 3136                   # anchors per chunk
NCH = 4                     # chunks per core
N_LOC = FC * NCH            # 12544
N_PAD = N_LOC * N_CORES     # 100352
FB = FC // 32               # 98 anchor-subblocks per chunk
NB = 2                      # ABC / RS double-buffer depth

F32 = mybir.dt.float32
F16 = mybir.dt.float16
COORD_SCALE = 1.0 / 16.0

LAST_EXEC_NS = None


def _ensure_axon_ntff_hook():
    try:
        import antenv.axon_hooks  # noqa: F401

        return
    except ImportError:
        pass
    import contextlib
    import ctypes
    import types

    import antenv

    m = types.ModuleType("antenv.axon_hooks")
    m._hook = None

    def set_axon_ntff_profile_hook(h):
        m._hook = h

    def get_axon_ntff_profile_hook():
        return m._hook

    m.set_axon_ntff_profile_hook = set_axon_ntff_profile_hook
    m.get_axon_ntff_profile_hook = get_axon_ntff_profile_hook
    sys.modules["antenv.axon_hooks"] = m
    antenv.axon_hooks = m

    so_path = os.environ.get("PJRT_LIBRARY_PATH", "/opt/axon/libaxon_pjrt.so")
    try:
        lib = ctypes.CDLL(so_path)
    except OSError:
        return
    if not hasattr(lib, "axon_start_nrt_profile"):
        return
    lib.axon_start_nrt_profile.argtypes = [
        ctypes.POINTER(ctypes.c_int64),
        ctypes.c_size_t,
    ]
    lib.axon_start_nrt_profile.restype = ctypes.c_int64
    lib.axon_stop_nrt_profile.argtypes = [ctypes.c_char_p]
    lib.axon_stop_nrt_profile.restype = ctypes.c_int64

    @contextlib.contextmanager
    def _hook(output_dir, device_ids):
        import jax

        jax.devices()
        if device_ids:
            ids = (ctypes.c_int64 * len(device_ids))(*device_ids)
            rc = lib.axon_start_nrt_profile(ids, len(device_ids))
        else:
            rc = lib.axon_start_nrt_profile(None, 0)
        if rc != 0:
            raise RuntimeError(f"axon_start_nrt_profile rc={rc}")
        try:
            yield
        finally:
            n = lib.axon_stop_nrt_profile(str(output_dir).encode())
            if n < 0:
                raise RuntimeError(f"axon_stop_nrt_profile rc={n}")

    set_axon_ntff_profile_hook(_hook)


def _patch_upload_artifacts():
    import concourse.bass_utils as bu

    if getattr(bu.upload_artifacts, "_safe", False):
        return
    orig = bu.upload_artifacts

    def safe(tmpdir):
        try:
            return orig(tmpdir)
        except Exception:
            return tmpdir

    safe._safe = True
    bu.upload_artifacts = safe


def _act_recip(scalar_eng, nc, out_ap, in_ap, bias=0.0, scale=1.0):
    """Emit Activation(Reciprocal); bias may be a [P,1] AP or a float.
    (The nc.scalar.activation wrapper rejects Reciprocal.)"""
    ins = [scalar_eng.lower_ap(in_ap)]
    if isinstance(bias, (int, float)):
        ins.append(mybir.ImmediateValue(dtype=F32, value=float(bias)))
    else:
        ins.append(scalar_eng.lower_ap(bias))
    ins.append(mybir.ImmediateValue(dtype=F32, value=float(scale)))
    ins.append(mybir.ImmediateValue(dtype=F32, value=0.0))
    return scalar_eng.add_instruction(
        mybir.InstActivation(
            name=nc.get_next_instruction_name(),
            func=mybir.ActivationFunctionType.Reciprocal,
            ins=ins,
            outs=[scalar_eng.lower_ap(out_ap)],
        )
    )


def _build_graph():
    nc = bass.Bass()
    AR_ext = nc.declare_dram_parameter("arows", [5, N_LOC], F16, isOutput=False)
    GC_ext = nc.declare_dram_parameter("gcols", [P, 10], F32, isOutput=False)
    out_ext = nc.declare_dram_parameter("out", [P, NCH * FB], F32, isOutput=True)

    Alu = mybir.AluOpType
    from contextlib import ExitStack

    with ExitStack() as _st:
        e = _st.enter_context

        ABC = e(nc.sbuf_tensor("ABC", [P, NB, 5, FC], F16))
        GCS = e(nc.sbuf_tensor("GCS", [P, 10], F32))
        # CHL: clip outputs [ax, gl, endp(lo,hi), FC]; its 8*FC region is
        # reused downstream: INT <- CHL[0,0], W <- CHL[0,1], R0/ST <- CHL[1,0],
        # tree scratch <- CHL[1,1] (the whole chain is serial on DVE).
        CHL = e(nc.sbuf_tensor("CHL", [P, 2, 2, 2, FC], F16))
        I = e(nc.sbuf_tensor("I", [P, 2, 2, FC], F16))
        RS = e(nc.sbuf_tensor("RS", [P, NB, 2, FC], F16))
        VOUT = e(nc.sbuf_tensor("VOUT", [P, NCH * FB], F16))
        R1 = e(nc.sbuf_tensor("R1", [P, NCH * FB], F16))
        MIOU = e(nc.sbuf_tensor("MIOU", [P, NCH * FB], F32))

        block = e(nc.Block())
        dma_sem = e(nc.semaphore("dma_sem"))
        dve_sem = e(nc.semaphore("dve_sem"))
        act_sem = e(nc.semaphore("act_sem"))

        def gcol(i):
            return GCS[:, i : i + 1]

        INT = CHL[:, 0, 0, :, :]          # [P, 2, FC]
        W = CHL[:, 0, 1, :, :]            # [P, 2, FC]
        R0 = CHL[:, 1, 0, 0, :]           # [P, FC]
        ST = CHL[:, 1, 0, 1, :]           # [P, FC]
        TREE = CHL[:, 1, 1, :, :].rearrange("p a b -> p (a b)")  # [P, 2*FC]

        @block.sync
        def _(sync):
            sync.dma_start(out=GCS[:, :], in_=GC_ext[:, :]).then_inc(dma_sem, 16)
            for c in range(NCH):
                if c >= NB:
                    sync.wait_ge(dve_sem, c - 1)
                    sync.wait_ge(act_sem, c - 1)
                a_ap = AR_ext[:, c * FC : (c + 1) * FC]
                a_b = bass.AP(
                    tensor=a_ap.tensor, offset=a_ap.offset,
                    ap=[[0, P]] + list(a_ap.ap),
                )
                sync.dma_start(out=ABC[:, c % NB, :, :], in_=a_b).then_inc(
                    dma_sem, 16
                )
            sync.wait_ge(dve_sem, NCH + 2)
            sync.dma_start(out=out_ext[:, :], in_=MIOU[:, :]).then_inc(dma_sem, 16)
            sync.wait_ge(dma_sem, 16 * (NCH + 2))

        @block.vector
        def _(vector):
            for c in range(NCH):
                cb = c % NB
                vector.wait_ge(dma_sem, 16 * (2 + c))
                # 4 clip TS 2-op @4x: CHL[ax,gl] = (a_rows max g1) min g2
                for ax in range(2):
                    rows = ABC[:, cb, 2 * ax : 2 * ax + 2, :]  # [P, 2, FC]
                    for gl in range(2):
                        ins = vector.tensor_scalar(
                            out=CHL[:, ax, gl, :, :], in0=rows,
                            scalar1=gcol(4 * gl + 2 * ax + 0),
                            scalar2=gcol(4 * gl + 2 * ax + 1),
                            op0=Alu.max, op1=Alu.min,
                        )
                ins.then_inc(dve_sem, 1)  # ABC[cb] consumed by DVE
                # I = hi - lo  [P, 2ax, 2gl, FC]
                vector.tensor_tensor(
                    out=I[:, :, :, :],
                    in0=CHL[:, :, :, 1, :],
                    in1=CHL[:, :, :, 0, :],
                    op=Alu.subtract,
                )
                # INT = Ix * Iy  [P, 2gl, FC]
                vector.tensor_tensor(
                    out=INT, in0=I[:, 0, :, :], in1=I[:, 1, :, :], op=Alu.mult
                )
                # W = INT * RS
                vector.wait_ge(act_sem, c + 1)
                vector.tensor_tensor(
                    out=W, in0=INT, in1=RS[:, cb, :, :], op=Alu.mult
                )
                # R0 = max over gl
                vector.tensor_tensor(
                    out=R0, in0=W[:, 0, :], in1=W[:, 1, :], op=Alu.max
                )
                # 32x32 block transpose puts gh innermost per anchor subblock
                vector.transpose(out=ST, in_=R0)
                # pairwise-max tree over gh: 32 -> 1
                src = ST.rearrange("p (fb g) -> p fb g", g=32)
                w = 16
                off = 0
                while w >= 2:
                    dst = TREE[:, off : off + FB * w].rearrange(
                        "p (fb g) -> p fb g", g=w
                    )
                    vector.tensor_tensor(
                        out=dst, in0=src[:, :, 0:w], in1=src[:, :, w : 2 * w],
                        op=Alu.max,
                    )
                    src = dst
                    off += FB * w
                    w //= 2
                last_tree = vector.tensor_tensor(
                    out=VOUT[:, c * FB : (c + 1) * FB].rearrange(
                        "p (fb g) -> p fb g", g=1
                    ),
                    in0=src[:, :, 0:1], in1=src[:, :, 1:2], op=Alu.max,
                )
            last_tree.then_inc(dve_sem, 1)  # tick NCH+1: VOUT complete
            vector.wait_ge(act_sem, NCH + 1)
            vector.tensor_tensor(
                out=MIOU[:, :], in0=VOUT[:, :], in1=R1[:, :], op=Alu.mult
            ).then_inc(dve_sem, 1)  # tick NCH+2

        @block.scalar
        def _(scalar):
            for c in range(NCH):
                cb = c % NB
                scalar.wait_ge(dma_sem, 16 * (2 + c))
                if c >= NB:
                    # W(c-2) has read RS[cb]: implied by clips(c-1) done
                    scalar.wait_ge(dve_sem, c)
                area = ABC[:, cb, 4, :]
                _act_recip(scalar, nc, RS[:, cb, 0, :], area, bias=gcol(8))
                _act_recip(
                    scalar, nc, RS[:, cb, 1, :], area, bias=gcol(9)
                ).then_inc(act_sem, 1)
            scalar.wait_ge(dve_sem, NCH + 1)
            _act_recip(
                scalar, nc, R1[:, :], VOUT[:, :], bias=1.0, scale=-1.0
            ).then_inc(act_sem, 1)

    return nc


def kernel(anchors: np.ndarray, gt_boxes: np.ndarray) -> np.ndarray:
    global LAST_EXEC_NS
    anchors = np.asarray(anchors, dtype=np.float32) * COORD_SCALE
    gt_boxes = np.asarray(gt_boxes, dtype=np.float32) * COORD_SCALE

    apad = np.zeros((N_PAD, 4), dtype=np.float32)
    apad[:N_ANCHORS] = anchors

    # gt scalar columns: partition p = b*32 + gh; g = gh*2 + gl
    g = gt_boxes.reshape(BATCH, N_GT, 4)
    gcols = np.zeros((P, 10), dtype=np.float32)
    bs = np.repeat(np.arange(BATCH), 32)
    gh = np.tile(np.arange(32), BATCH)
    for gl in range(2):
        gg = g[bs, gh * 2 + gl]          # [128, 4] (x1,y1,x2,y2)
        gcols[:, 4 * gl + 0] = gg[:, 0]  # gx1
        gcols[:, 4 * gl + 1] = gg[:, 2]  # gx2
        gcols[:, 4 * gl + 2] = gg[:, 1]  # gy1
        gcols[:, 4 * gl + 3] = gg[:, 3]  # gy2
        gcols[:, 8 + gl] = (gg[:, 2] - gg[:, 0]) * (gg[:, 3] - gg[:, 1])

    in_maps = []
    for c in range(N_CORES):
        sh = apad[c * N_LOC : (c + 1) * N_LOC]
        arows = np.empty((5, N_LOC), dtype=np.float16)
        arows[0] = sh[:, 0]  # ax1
        arows[1] = sh[:, 2]  # ax2
        arows[2] = sh[:, 1]  # ay1
        arows[3] = sh[:, 3]  # ay2
        arows[4] = (sh[:, 2] - sh[:, 0]) * (sh[:, 3] - sh[:, 1])
        in_maps.append({"arows": np.ascontiguousarray(arows), "gcols": gcols})

    nc = _build_graph()
    trace = os.environ.get("ANCHOR_TRACE", "0") == "1"
    core_ids = list(range(N_CORES))
    if trace:
        _ensure_axon_ntff_hook()
        _patch_upload_artifacts()
        try:
            res = run_bass_kernel_spmd(nc, in_maps, core_ids=core_ids, trace=True)
        except Exception as e:
            print(
                f"trace run failed ({type(e).__name__}: {e}); falling back",
                file=sys.stderr,
            )
            res = run_bass_kernel_spmd(nc, in_maps, core_ids=core_ids, trace=False)
    else:
        res = run_bass_kernel_spmd(nc, in_maps, core_ids=core_ids, trace=False)
    LAST_EXEC_NS = res.exec_time_ns

    out = np.empty((BATCH, N_PAD), dtype=np.float32)
    for c in range(N_CORES):
        o = res.results[c]["out"].reshape(BATCH, 32, NCH * FB)
        # value(b, i, fb) is anchor a = fb*32 + i
        out[:, c * N_LOC : (c + 1) * N_LOC] = o.transpose(0, 2, 1).reshape(
            BATCH, N_LOC
        )
    return out[:, :N_ANCHORS]


# revision 18
# speedup vs baseline: 1.4601x; 1.0033x over previous
"""AnchorTargetLayer max-IoU kernel for 8 TRN2 NeuronCores (v7).

max_iou[b, n] = max_g IoU(anchor_n, gt_box[b, g]);
anchors [100000, 4] f32, gt_boxes [4, 64, 4] f32 -> out [4, 100000] f32.

Sharding: anchors split 8 ways (12544/core incl pad), gt replicated, no
collectives.

v7 layout (vs v5's anchors-on-partitions): (batch, gt) pairs live on
PARTITIONS — p = b*32 + gh, gl in {0,1} on the free dim (g = gh*2+gl) —
and ANCHORS live on the FREE dim, broadcast to all 128 partitions by a
partition-stride-0 DMA (4.1MB/chunk, ~10.4us, fully overlapped). This
turns v5's 392 narrow (256-wide) tensor_scalar ops (~360ns each, fixed
overhead dominated) into a handful of wide (3136) ops per chunk.

Per chunk (Fc=3136 anchors, 4 chunks/core), all fp16:
  clip:  CHL[ax,gl] = clamp(a_lo/a_hi rows; g1[p], g2[p])   4 TS 2-op @4x
  span:  I = CHL[...,hi] - CHL[...,lo]  (>=0 by clipping)   1 wide TT
  int:   INT = Ix * Iy                                      1 TT
  w:     W = INT * RS,  RS = 1/(aarea + garea) from ACT     1 TT
  glmax: R0 = max(W[gl0], W[gl1])                           1 TT
  gmax over gh=32 (partitions): stream_transpose 32x32 puts gh
         innermost in the free dim, then a 5-step pairwise TT-max tree.
Final: iou = v / (1 - v)  (w = int/(areaA+areaG) is monotone in iou).

Engine split: DVE ~29us/chunk; ACT runs the two reciprocals per chunk
(bias = garea per partition, fused add) ~6us; DMA broadcast ~10.4us.
SBUF: CHL block is reused for INT/W/R0/ST/tree scratch (serial chain).
"""

import os
import sys

import numpy as np

sys.path.insert(0, "/opt/trn_rl_repo")

import concourse.bass as bass
import concourse.mybir as mybir
from concourse.bass_utils import run_bass_kernel_spmd

N_ANCHORS = 100000
BATCH = 4
N_GT = 64
N_CORES = 8

P = 128
FC = 3136                   # max anchors per chunk
# small head chunks for fast pipeline start; small tail chunk shortens drain
CHUNKS = [1024, 2112, 3136, 3136, 2112, 1024]
NCH = len(CHUNKS)
N_LOC = sum(CHUNKS)         # 12544
N_PAD = N_LOC * N_CORES     # 100352
NB = 2                      # ABC / RS double-buffer depth

F32 = mybir.dt.float32
F16 = mybir.dt.float16
COORD_SCALE = 1.0 / 16.0

LAST_EXEC_NS = None


def _ensure_axon_ntff_hook():
    try:
        import antenv.axon_hooks  # noqa: F401

        return
    except ImportError:
        pass
    import contextlib
    import ctypes
    import types

    import antenv

    m = types.ModuleType("antenv.axon_hooks")
    m._hook = None

    def set_axon_ntff_profile_hook(h):
        m._hook = h

    def get_axon_ntff_profile_hook():
        return m._hook

    m.set_axon_ntff_profile_hook = set_axon_ntff_profile_hook
    m.get_axon_ntff_profile_hook = get_axon_ntff_profile_hook
    sys.modules["antenv.axon_hooks"] = m
    antenv.axon_hooks = m

    so_path = os.environ.get("PJRT_LIBRARY_PATH", "/opt/axon/libaxon_pjrt.so")
    try:
        lib = ctypes.CDLL(so_path)
    except OSError:
        return
    if not hasattr(lib, "axon_start_nrt_profile"):
        return
    lib.axon_start_nrt_profile.argtypes = [
        ctypes.POINTER(ctypes.c_int64),
        ctypes.c_size_t,
    ]
    lib.axon_start_nrt_profile.restype = ctypes.c_int64
    lib.axon_stop_nrt_profile.argtypes = [ctypes.c_char_p]
    lib.axon_stop_nrt_profile.restype = ctypes.c_int64

    @contextlib.contextmanager
    def _hook(output_dir, device_ids):
        import jax

        jax.devices()
        if device_ids:
            ids = (ctypes.c_int64 * len(device_ids))(*device_ids)
            rc = lib.axon_start_nrt_profile(ids, len(device_ids))
        else:
            rc = lib.axon_start_nrt_profile(None, 0)
        if rc != 0:
            raise RuntimeError(f"axon_start_nrt_profile rc={rc}")
        try:
            yield
        finally:
            n = lib.axon_stop_nrt_profile(str(output_dir).encode())
            if n < 0:
                raise RuntimeError(f"axon_stop_nrt_profile rc={n}")

    set_axon_ntff_profile_hook(_hook)


def _patch_upload_artifacts():
    import concourse.bass_utils as bu

    if getattr(bu.upload_artifacts, "_safe", False):
        return
    orig = bu.upload_artifacts

    def safe(tmpdir):
        try:
            return orig(tmpdir)
        except Exception:
            return tmpdir

    safe._safe = True
    bu.upload_artifacts = safe


def _act_recip(scalar_eng, nc, out_ap, in_ap, bias=0.0, scale=1.0):
    """Emit Activation(Reciprocal); bias may be a [P,1] AP or a float.
    (The nc.scalar.activation wrapper rejects Reciprocal.)"""
    ins = [scalar_eng.lower_ap(in_ap)]
    if isinstance(bias, (int, float)):
        ins.append(mybir.ImmediateValue(dtype=F32, value=float(bias)))
    else:
        ins.append(scalar_eng.lower_ap(bias))
    ins.append(mybir.ImmediateValue(dtype=F32, value=float(scale)))
    ins.append(mybir.ImmediateValue(dtype=F32, value=0.0))
    return scalar_eng.add_instruction(
        mybir.InstActivation(
            name=nc.get_next_instruction_name(),
            func=mybir.ActivationFunctionType.Reciprocal,
            ins=ins,
            outs=[scalar_eng.lower_ap(out_ap)],
        )
    )


def _build_graph():
    nc = bass.Bass()
    AR_ext = nc.declare_dram_parameter("arows", [5, N_LOC], F16, isOutput=False)
    GC_ext = nc.declare_dram_parameter("gcols", [P, 10], F32, isOutput=False)
    NFB = N_LOC // 32
    out_ext = nc.declare_dram_parameter("out", [P, NFB], F32, isOutput=True)

    Alu = mybir.AluOpType
    from contextlib import ExitStack

    with ExitStack() as _st:
        e = _st.enter_context

        ABC = e(nc.sbuf_tensor("ABC", [P, NB, 5, FC], F16))
        GCS = e(nc.sbuf_tensor("GCS", [P, 10], F32))
        # CHL: clip outputs [ax, gl, endp(lo,hi), FC]; its 8*FC region is
        # reused downstream: INT <- CHL[0,0], W <- CHL[0,1], R0/ST <- CHL[1,0],
        # tree scratch <- CHL[1,1] (the whole chain is serial on DVE).
        CHL = e(nc.sbuf_tensor("CHL", [P, 2, 2, 2, FC], F16))
        I = e(nc.sbuf_tensor("I", [P, 2, 2, FC], F16))
        RS = e(nc.sbuf_tensor("RS", [P, NB, 2, FC], F16))
        VOUT = e(nc.sbuf_tensor("VOUT", [P, NFB], F16))
        R1 = e(nc.sbuf_tensor("R1", [P, NFB], F16))
        MIOU = e(nc.sbuf_tensor("MIOU", [P, NFB], F32))

        block = e(nc.Block())
        dma_sem = e(nc.semaphore("dma_sem"))
        dve_sem = e(nc.semaphore("dve_sem"))
        miou_sem = e(nc.semaphore("miou_sem"))
        act_rs = e(nc.semaphore("act_rs"))
        act_r1 = e(nc.semaphore("act_r1"))

        def gcol(i):
            return GCS[:, i : i + 1]

        offs = [sum(CHUNKS[:i]) for i in range(NCH)]
        fbo = [o // 32 for o in offs]

        # dve_sem ticks: 2c+1 = clips(c) done, 2c+2 = tree(c) done,
        # 2*NCH+1 = last MIOU done.
        @block.sync
        def _(sync):
            for c in range(NCH):
                fc = CHUNKS[c]
                if c >= NB:
                    sync.wait_ge(dve_sem, 2 * c - 3)
                    sync.wait_ge(act_rs, c - 1)
                a_ap = AR_ext[:, offs[c] : offs[c] + fc]
                a_b = bass.AP(
                    tensor=a_ap.tensor, offset=a_ap.offset,
                    ap=[[0, P]] + list(a_ap.ap),
                )
                sync.dma_start(out=ABC[:, c % NB, :, 0:fc], in_=a_b).then_inc(
                    dma_sem, 16
                )
                if c == 0:
                    sync.dma_start(out=GCS[:, :], in_=GC_ext[:, :]).then_inc(
                        dma_sem, 16
                    )
            # per-chunk output drains as soon as each MIOU slab is written
            for c in range(NCH):
                sync.wait_ge(miou_sem, c + 1)
                sync.dma_start(
                    out=out_ext[:, fbo[c] : fbo[c] + CHUNKS[c] // 32],
                    in_=MIOU[:, fbo[c] : fbo[c] + CHUNKS[c] // 32],
                ).then_inc(dma_sem, 16)
            sync.wait_ge(dma_sem, 16 * (2 * NCH + 1))

        @block.vector
        def _(vector):
            for c in range(NCH):
                cb = c % NB
                fc = CHUNKS[c]
                fb = fc // 32
                vector.wait_ge(dma_sem, 16 * (2 + c))
                # 4 clip TS 2-op @4x: CHL[ax,gl] = (a_rows max g1) min g2
                for ax in range(2):
                    rows = ABC[:, cb, 2 * ax : 2 * ax + 2, 0:fc]  # [P, 2, fc]
                    for gl in range(2):
                        clip_ins = vector.tensor_scalar(
                            out=CHL[:, ax, gl, :, 0:fc], in0=rows,
                            scalar1=gcol(4 * gl + 2 * ax + 0),
                            scalar2=gcol(4 * gl + 2 * ax + 1),
                            op0=Alu.max, op1=Alu.min,
                        )
                clip_ins.then_inc(dve_sem, 1)  # tick 2c+1
                # fixup of the previous chunk rides here (ACT slack)
                if c >= 1:
                    vector.wait_ge(act_r1, c)
                    pc = c - 1
                    vector.tensor_tensor(
                        out=MIOU[:, fbo[pc] : fbo[pc] + CHUNKS[pc] // 32],
                        in0=VOUT[:, fbo[pc] : fbo[pc] + CHUNKS[pc] // 32],
                        in1=R1[:, fbo[pc] : fbo[pc] + CHUNKS[pc] // 32],
                        op=Alu.mult,
                    ).then_inc(miou_sem, 1)
                # I = hi - lo  [P, 2ax, 2gl, fc]
                vector.tensor_tensor(
                    out=I[:, :, :, 0:fc],
                    in0=CHL[:, :, :, 1, 0:fc],
                    in1=CHL[:, :, :, 0, 0:fc],
                    op=Alu.subtract,
                )
                # INT = Ix * Iy  [P, 2gl, fc]
                vector.tensor_tensor(
                    out=CHL[:, 0, 0, :, 0:fc],
                    in0=I[:, 0, :, 0:fc], in1=I[:, 1, :, 0:fc], op=Alu.mult,
                )
                # W = INT * RS
                vector.wait_ge(act_rs, c + 1)
                vector.tensor_tensor(
                    out=CHL[:, 0, 1, :, 0:fc],
                    in0=CHL[:, 0, 0, :, 0:fc], in1=RS[:, cb, :, 0:fc],
                    op=Alu.mult,
                )
                # R0 = max over gl
                vector.tensor_tensor(
                    out=CHL[:, 1, 0, 0, 0:fc],
                    in0=CHL[:, 0, 1, 0, 0:fc], in1=CHL[:, 0, 1, 1, 0:fc],
                    op=Alu.max,
                )
                # 32x32 block transpose puts gh innermost per anchor subblock
                vector.transpose(
                    out=CHL[:, 1, 0, 1, 0:fc], in_=CHL[:, 1, 0, 0, 0:fc]
                )
                # pairwise-max tree over gh: 32 -> 1
                TREE = CHL[:, 1, 1, :, :].rearrange("p a b -> p (a b)")
                src = CHL[:, 1, 0, 1, 0:fc].rearrange("p (fb g) -> p fb g", g=32)
                w = 16
                off = 0
                while w >= 2:
                    dst = TREE[:, off : off + fb * w].rearrange(
                        "p (fb g) -> p fb g", g=w
                    )
                    vector.tensor_tensor(
                        out=dst, in0=src[:, :, 0:w], in1=src[:, :, w : 2 * w],
                        op=Alu.max,
                    )
                    src = dst
                    off += fb * w
                    w //= 2
                vector.tensor_tensor(
                    out=VOUT[:, fbo[c] : fbo[c] + fb].rearrange(
                        "p (fb g) -> p fb g", g=1
                    ),
                    in0=src[:, :, 0:1], in1=src[:, :, 1:2], op=Alu.max,
                ).then_inc(dve_sem, 1)  # tick 2c+2
            # last chunk fixup
            c = NCH - 1
            vector.wait_ge(act_r1, NCH)
            vector.tensor_tensor(
                out=MIOU[:, fbo[c] : fbo[c] + CHUNKS[c] // 32],
                in0=VOUT[:, fbo[c] : fbo[c] + CHUNKS[c] // 32],
                in1=R1[:, fbo[c] : fbo[c] + CHUNKS[c] // 32],
                op=Alu.mult,
            ).then_inc(miou_sem, 1)

        @block.scalar
        def _(scalar):
            for c in range(NCH):
                cb = c % NB
                fc = CHUNKS[c]
                scalar.wait_ge(dma_sem, 16 * (2 + c))
                if c >= NB:
                    # W(c-2) has read RS[cb]: implied by clips(c-1) done
                    scalar.wait_ge(dve_sem, 2 * c - 1)
                area = ABC[:, cb, 4, 0:fc]
                _act_recip(scalar, nc, RS[:, cb, 0, 0:fc], area, bias=gcol(8))
                _act_recip(
                    scalar, nc, RS[:, cb, 1, 0:fc], area, bias=gcol(9)
                ).then_inc(act_rs, 1)
                # R1 fixup for an earlier finished chunk: interleave to avoid
                # stalling the recips; chunk c-1's tree is done by now or soon
                if c >= 1:
                    pc = c - 1
                    scalar.wait_ge(dve_sem, 2 * pc + 2)
                    _act_recip(
                        scalar, nc,
                        R1[:, fbo[pc] : fbo[pc] + CHUNKS[pc] // 32],
                        VOUT[:, fbo[pc] : fbo[pc] + CHUNKS[pc] // 32],
                        bias=1.0, scale=-1.0,
                    ).then_inc(act_r1, 1)
            c = NCH - 1
            scalar.wait_ge(dve_sem, 2 * c + 2)
            _act_recip(
                scalar, nc, R1[:, fbo[c] : fbo[c] + CHUNKS[c] // 32],
                VOUT[:, fbo[c] : fbo[c] + CHUNKS[c] // 32],
                bias=1.0, scale=-1.0,
            ).then_inc(act_r1, 1)

    return nc


def kernel(anchors: np.ndarray, gt_boxes: np.ndarray) -> np.ndarray:
    global LAST_EXEC_NS
    anchors = np.asarray(anchors, dtype=np.float32) * COORD_SCALE
    gt_boxes = np.asarray(gt_boxes, dtype=np.float32) * COORD_SCALE

    apad = np.zeros((N_PAD, 4), dtype=np.float32)
    apad[:N_ANCHORS] = anchors

    # gt scalar columns: partition p = b*32 + gh; g = gh*2 + gl
    g = gt_boxes.reshape(BATCH, N_GT, 4)
    gcols = np.zeros((P, 10), dtype=np.float32)
    bs = np.repeat(np.arange(BATCH), 32)
    gh = np.tile(np.arange(32), BATCH)
    for gl in range(2):
        gg = g[bs, gh * 2 + gl]          # [128, 4] (x1,y1,x2,y2)
        gcols[:, 4 * gl + 0] = gg[:, 0]  # gx1
        gcols[:, 4 * gl + 1] = gg[:, 2]  # gx2
        gcols[:, 4 * gl + 2] = gg[:, 1]  # gy1
        gcols[:, 4 * gl + 3] = gg[:, 3]  # gy2
        gcols[:, 8 + gl] = (gg[:, 2] - gg[:, 0]) * (gg[:, 3] - gg[:, 1])

    in_maps = []
    for c in range(N_CORES):
        sh = apad[c * N_LOC : (c + 1) * N_LOC]
        arows = np.empty((5, N_LOC), dtype=np.float16)
        arows[0] = sh[:, 0]  # ax1
        arows[1] = sh[:, 2]  # ax2
        arows[2] = sh[:, 1]  # ay1
        arows[3] = sh[:, 3]  # ay2
        arows[4] = (sh[:, 2] - sh[:, 0]) * (sh[:, 3] - sh[:, 1])
        in_maps.append({"arows": np.ascontiguousarray(arows), "gcols": gcols})

    nc = _build_graph()
    trace = os.environ.get("ANCHOR_TRACE", "0") == "1"
    core_ids = list(range(N_CORES))
    if trace:
        _ensure_axon_ntff_hook()
        _patch_upload_artifacts()
        try:
            res = run_bass_kernel_spmd(nc, in_maps, core_ids=core_ids, trace=True)
        except Exception as e:
            print(
                f"trace run failed ({type(e).__name__}: {e}); falling back",
                file=sys.stderr,
            )
            res = run_bass_kernel_spmd(nc, in_maps, core_ids=core_ids, trace=False)
    else:
        res = run_bass_kernel_spmd(nc, in_maps, core_ids=core_ids, trace=False)
    LAST_EXEC_NS = res.exec_time_ns

    out = np.empty((BATCH, N_PAD), dtype=np.float32)
    for c in range(N_CORES):
        o = res.results[c]["out"].reshape(BATCH, 32, N_LOC // 32)
        # value(b, i, fb) is anchor a = fb*32 + i
        out[:, c * N_LOC : (c + 1) * N_LOC] = o.transpose(0, 2, 1).reshape(
            BATCH, N_LOC
        )
    return out[:, :N_ANCHORS]
